# revision 3
# baseline (speedup 1.0000x reference)
"""Trainium2 Bass kernel for 2-layer GraphSAGE (mean aggregation).

Strategy (8-core SPMD, nodes sharded 12500/core):
- Host: sorts/pads each core's in-edges into fixed 128-edge tiles aligned to
  128-dst-node windows (uniform tile count across cores so one SPMD program
  works), pre-gathers layer-1 messages x[src] (input reindexing) and bakes
  1/deg into per-edge weights.
- Device layer 1: stream pre-gathered messages, segment-sum via one-hot
  indicator matmuls (M[e,r] = (dst_e==r)*w_e built on DVE from an iota tile),
  PSUM-accumulated per 512-node window, then W1l/W1r matmuls + bias + ReLU in
  [feat, node] orientation.
- h1 transposed to row layout via PE-identity matmuls, AllGather -> full
  [100352, 64] bf16 table per core.
- Device layer 2: per-tile indirect-DMA gather of h1 rows, same one-hot
  aggregation, W2l/W2r matmuls + bias, transpose back, DMA out fp32.
"""
import sys

sys.path.insert(0, '/opt/trn_rl_repo')
import numpy as np
import ml_dtypes

BF16 = ml_dtypes.bfloat16
N = 100000
D = 64
NCORES = 8
NLOC = N // NCORES          # 12500
P = 128
NW = (NLOC + P - 1) // P    # 98 dst windows per core
WROWS = NW * P              # 12544 padded local rows
TBL_ROWS = NCORES * WROWS   # 100352 rows in the gathered h1 table


def _layout_row(n):
    """Row index of global node n inside the AllGather'd h1 table."""
    c = n // NLOC
    r = n % NLOC
    t = r // P
    p = r % P
    return c * WROWS + p * NW + t


def _prep_core(c, src, dst, inv, x, K):
    """Slot edges of core c into NW*K tiles of 128, window-aligned."""
    m = (dst >= c * NLOC) & (dst < (c + 1) * NLOC)
    es, ed = src[m], dst[m] - c * NLOC
    w = inv[dst[m]]
    win = ed // P
    order = np.argsort(win, kind='stable')
    es, ed, w, win = es[order], ed[order], w[order], win[order]

    T = NW * K
    slots_src = np.zeros(T * P, dtype=np.int64)
    slots_dstloc = np.full(T * P, -1.0, dtype=np.float32)
    slots_w = np.zeros(T * P, dtype=np.float32)
    # fill window-by-window
    counts = np.bincount(win, minlength=NW)
    starts = np.concatenate([[0], np.cumsum(counts)[:-1]])
    for wi in range(NW):
        cnt = counts[wi]
        base = wi * K * P
        sl = slice(starts[wi], starts[wi] + cnt)
        slots_src[base:base + cnt] = es[sl]
        slots_dstloc[base:base + cnt] = (ed[sl] % P).astype(np.float32)
        slots_w[base:base + cnt] = w[sl]

    # [T*P] slot-major (tile t, partition p = slot t*P+p) -> [P, T] arrays
    def to_pt(a, dt):
        return np.ascontiguousarray(a.reshape(T, P).T.astype(dt))

    dstloc_pt = to_pt(slots_dstloc, np.float32)
    w_pt = to_pt(slots_w, np.float32)
    src2_pt = to_pt(_layout_row(slots_src), np.int32)
    # pre-gathered layer-1 messages, bf16, [P, T*64] partition-major
    msgs = x[slots_src].astype(BF16)           # [T*P, 64]
    msgs_pt = np.ascontiguousarray(
        msgs.reshape(T, P, D).transpose(1, 0, 2).reshape(P, T * D))
    # local x^T padded to WROWS cols
    xT = np.zeros((D, WROWS), dtype=BF16)
    xT[:, :NLOC] = x[c * NLOC:(c + 1) * NLOC].T.astype(BF16)
    return msgs_pt, dstloc_pt, w_pt, src2_pt, xT


def _build_program(K):
    import concourse.bass as bass
    import concourse.tile as tile
    from concourse import bacc, mybir

    T = NW * K
    nc = bacc.Bacc("TRN2", target_bir_lowering=False, debug=False,
                   num_devices=NCORES)
    dt = mybir.dt

    msgs_d = nc.dram_tensor("msgs", [P, T * D], dt.bfloat16, kind="ExternalInput")
    dstloc_d = nc.dram_tensor("dstloc", [P, T], dt.float32, kind="ExternalInput")
    wts_d = nc.dram_tensor("wts", [P, T], dt.float32, kind="ExternalInput")
    src2_d = nc.dram_tensor("src2", [P, T], dt.int32, kind="ExternalInput")
    xT_d = nc.dram_tensor("xT", [D, WROWS], dt.bfloat16, kind="ExternalInput")
    iota_d = nc.dram_tensor("iota", [P, P], dt.bfloat16, kind="ExternalInput")
    id64_d = nc.dram_tensor("id64", [D, D], dt.bfloat16, kind="ExternalInput")
    id64f_d = nc.dram_tensor("id64f", [D, D], dt.float32, kind="ExternalInput")
    w1l_d = nc.dram_tensor("w1lT", [D, D], dt.bfloat16, kind="ExternalInput")
    w1r_d = nc.dram_tensor("w1rT", [D, D], dt.bfloat16, kind="ExternalInput")
    w2l_d = nc.dram_tensor("w2lT", [D, D], dt.bfloat16, kind="ExternalInput")
    w2r_d = nc.dram_tensor("w2rT", [D, D], dt.bfloat16, kind="ExternalInput")
    b1_d = nc.dram_tensor("b1c", [D, 1], dt.float32, kind="ExternalInput")
    b2_d = nc.dram_tensor("b2c", [D, 1], dt.float32, kind="ExternalInput")
    out_d = nc.dram_tensor("out", [WROWS, D], dt.float32, kind="ExternalOutput")

    # supers: groups of up to 4 windows sharing one [64,512] psum bank
    supers = []
    wi = 0
    while wi < NW:
        sw = min(4, NW - wi)
        supers.append((wi, sw))
        wi += sw

    CHUNK_W = 14  # windows of msgs per streamed chunk
    with tile.TileContext(nc) as tc:
        with (
            tc.tile_pool(name="const", bufs=1) as cpool,
            tc.tile_pool(name="chunks", bufs=2) as chpool,
            tc.tile_pool(name="mtiles", bufs=12) as mpool,
            tc.tile_pool(name="gtiles", bufs=24) as gpool,
            tc.tile_pool(name="small", bufs=3) as spool,
            tc.tile_pool(name="psA", bufs=2, space="PSUM") as psA,
            tc.tile_pool(name="psB", bufs=2, space="PSUM") as psB,
            tc.tile_pool(name="psT", bufs=2, space="PSUM") as psT,
            tc.tile_pool(name="dram", bufs=1, space="DRAM") as dpool,
        ):
            # resident SBUF state
            dstloc_sb = cpool.tile([P, T], dt.float32, tag="dstloc")
            wts_sb = cpool.tile([P, T], dt.float32, tag="wts")
            src2_sb = cpool.tile([P, T], dt.int32, tag="src2")
            xT_sb = cpool.tile([D, WROWS], dt.bfloat16, tag="xT")
            iota_sb = cpool.tile([P, P], dt.bfloat16, tag="iota")
            id64_sb = cpool.tile([D, D], dt.bfloat16, tag="id64")
            id64f_sb = cpool.tile([D, D], dt.float32, tag="id64f")
            w1l_sb = cpool.tile([D, D], dt.bfloat16, tag="w1l")
            w1r_sb = cpool.tile([D, D], dt.bfloat16, tag="w1r")
            w2l_sb = cpool.tile([D, D], dt.bfloat16, tag="w2l")
            w2r_sb = cpool.tile([D, D], dt.bfloat16, tag="w2r")
            b1_sb = cpool.tile([D, 1], dt.float32, tag="b1")
            b2_sb = cpool.tile([D, 1], dt.float32, tag="b2")
            h1T_sb = cpool.tile([D, WROWS], dt.bfloat16, tag="h1T")
            h1rows_sb = cpool.tile([P, NW * D], dt.bfloat16, tag="h1rows")

            for t_sb, t_d in [(dstloc_sb, dstloc_d), (wts_sb, wts_d),
                              (src2_sb, src2_d), (xT_sb, xT_d),
                              (iota_sb, iota_d), (id64_sb, id64_d),
                              (id64f_sb, id64f_d),
                              (w1l_sb, w1l_d), (w1r_sb, w1r_d),
                              (w2l_sb, w2l_d), (w2r_sb, w2r_d),
                              (b1_sb, b1_d), (b2_sb, b2_d)]:
                nc.sync.dma_start(out=t_sb[:], in_=t_d.ap())

            h1loc_dram = dpool.tile([WROWS, D], dt.bfloat16, tag="h1loc")
            h1full_dram = dpool.tile([TBL_ROWS, D], dt.bfloat16, tag="h1full")

            # ---------------- layer 1 ----------------
            nchunks = (NW + CHUNK_W - 1) // CHUNK_W
            chunk_tiles = {}
            for ci in range(nchunks):
                w0 = ci * CHUNK_W
                nw = min(CHUNK_W, NW - w0)
                ch = chpool.tile([P, CHUNK_W * K * D], dt.bfloat16, tag="msgs")
                nc.sync.dma_start(
                    out=ch[:, :nw * K * D],
                    in_=msgs_d.ap()[:, w0 * K * D:(w0 + nw) * K * D])
                chunk_tiles[ci] = ch

            for w0, sw in supers:
                agg_ps = psA.tile([D, 4 * P], dt.float32, tag="agg")
                for s in range(sw):
                    wi = w0 + s
                    ci, woff = wi // CHUNK_W, wi % CHUNK_W
                    ch = chunk_tiles[ci]
                    for k in range(K):
                        t = wi * K + k
                        mt = mpool.tile([P, P], dt.bfloat16, tag="M")
                        # Pool is idle in layer 1 (no gathers yet): split the
                        # one-hot builds across both vector-capable engines.
                        veng = nc.vector if (t & 1) else nc.gpsimd
                        veng.tensor_scalar(
                            out=mt[:], in0=iota_sb[:],
                            scalar1=dstloc_sb[:, t:t + 1],
                            scalar2=wts_sb[:, t:t + 1],
                            op0=mybir.AluOpType.is_equal,
                            op1=mybir.AluOpType.mult)
                        nc.tensor.matmul(
                            out=agg_ps[:, s * P:(s + 1) * P],
                            lhsT=ch[:, (woff * K + k) * D:(woff * K + k + 1) * D],
                            rhs=mt[:], start=(k == 0), stop=(k == K - 1))
                agg_sb = spool.tile([D, 4 * P], dt.bfloat16, tag="aggsb")
                nc.vector.tensor_copy(out=agg_sb[:, :sw * P],
                                      in_=agg_ps[:, :sw * P])
                h_ps = psB.tile([D, 4 * P], dt.float32, tag="hps")
                nc.tensor.matmul(out=h_ps[:, :sw * P], lhsT=w1l_sb[:],
                                 rhs=agg_sb[:, :sw * P], start=True, stop=False)
                nc.tensor.matmul(out=h_ps[:, :sw * P], lhsT=w1r_sb[:],
                                 rhs=xT_sb[:, w0 * P:(w0 + sw) * P],
                                 start=False, stop=True)
                nc.scalar.activation(
                    out=h1T_sb[:, w0 * P:(w0 + sw) * P], in_=h_ps[:, :sw * P],
                    func=mybir.ActivationFunctionType.Relu, bias=b1_sb[:])

            # transpose h1T -> rows, stage, one DMA to DRAM, AllGather
            for wi in range(NW):
                tp = psT.tile([P, D], dt.float32, tag="tp")
                nc.tensor.matmul(out=tp[:], lhsT=h1T_sb[:, wi * P:(wi + 1) * P],
                                 rhs=id64_sb[:], start=True, stop=True)
                nc.vector.tensor_copy(out=h1rows_sb[:, wi * D:(wi + 1) * D],
                                      in_=tp[:])
            nc.sync.dma_start(
                out=h1loc_dram[:].rearrange("(p t) f -> p (t f)", p=P),
                in_=h1rows_sb[:])
            nc.gpsimd.collective_compute(
                "AllGather", mybir.AluOpType.bypass,
                replica_groups=[list(range(NCORES))],
                ins=[h1loc_dram[:]], outs=[h1full_dram[:]])

            # ---------------- layer 2 ----------------
            out2T_sb = cpool.tile([D, WROWS], dt.float32, tag="out2T")
            for w0, sw in supers:
                agg_ps = psA.tile([D, 4 * P], dt.float32, tag="agg")
                for s in range(sw):
                    wi = w0 + s
                    for k in range(K):
                        t = wi * K + k
                        gt = gpool.tile([P, D], dt.bfloat16, tag="g")
                        nc.gpsimd.indirect_dma_start(
                            out=gt[:], out_offset=None, in_=h1full_dram[:],
                            in_offset=bass.IndirectOffsetOnAxis(
                                ap=src2_sb[:, t:t + 1], axis=0))
                        mt = mpool.tile([P, P], dt.bfloat16, tag="M")
                        nc.vector.tensor_scalar(
                            out=mt[:], in0=iota_sb[:],
                            scalar1=dstloc_sb[:, t:t + 1],
                            scalar2=wts_sb[:, t:t + 1],
                            op0=mybir.AluOpType.is_equal,
                            op1=mybir.AluOpType.mult)
                        nc.tensor.matmul(
                            out=agg_ps[:, s * P:(s + 1) * P], lhsT=gt[:],
                            rhs=mt[:], start=(k == 0), stop=(k == K - 1))
                agg_sb = spool.tile([D, 4 * P], dt.bfloat16, tag="aggsb")
                nc.vector.tensor_copy(out=agg_sb[:, :sw * P],
                                      in_=agg_ps[:, :sw * P])
                h_ps = psB.tile([D, 4 * P], dt.float32, tag="hps")
                nc.tensor.matmul(out=h_ps[:, :sw * P], lhsT=w2l_sb[:],
                                 rhs=agg_sb[:, :sw * P], start=True, stop=False)
                nc.tensor.matmul(out=h_ps[:, :sw * P], lhsT=w2r_sb[:],
                                 rhs=h1T_sb[:, w0 * P:(w0 + sw) * P],
                                 start=False, stop=True)
                nc.vector.tensor_scalar_add(
                    out=out2T_sb[:, w0 * P:(w0 + sw) * P],
                    in0=h_ps[:, :sw * P], scalar1=b2_sb[:])

            # transpose out2T -> rows -> DRAM
            outv = out_d.ap().rearrange("(p t) f -> p t f", p=P)
            for wi in range(NW):
                tp = psT.tile([P, D], dt.float32, tag="tp")
                nc.tensor.matmul(out=tp[:], lhsT=out2T_sb[:, wi * P:(wi + 1) * P],
                                 rhs=id64f_sb[:], start=True, stop=True)
                ot = spool.tile([P, D], dt.float32, tag="orow")
                nc.vector.tensor_copy(out=ot[:], in_=tp[:])
                nc.sync.dma_start(out=outv[:, wi, :], in_=ot[:])

    nc.compile()
    return nc


def kernel(x, edge_index, W1l, W1r, b1, W2l, W2r, b2):
    from concourse import bass_utils

    x = np.asarray(x, dtype=np.float32)
    edge_index = np.asarray(edge_index)
    src = edge_index[0].astype(np.int64)
    dst = edge_index[1].astype(np.int64)
    cnt = np.bincount(dst, minlength=N).astype(np.float32)
    inv = (1.0 / np.maximum(cnt, 1.0)).astype(np.float32)

    # uniform tile count across cores/windows
    wid = (dst % NLOC) // P + (dst // NLOC) * NW
    wc = np.bincount(wid, minlength=NCORES * NW)
    K = int(np.max((wc + P - 1) // P))
    K = max(K, 1)

    iota = np.tile(np.arange(P, dtype=np.float32), (P, 1)).astype(BF16)
    id64 = np.eye(D, dtype=np.float32)
    common = {
        "iota": iota, "id64": id64.astype(BF16), "id64f": id64,
        "w1lT": np.asarray(W1l, np.float32).T.astype(BF16).copy(),
        "w1rT": np.asarray(W1r, np.float32).T.astype(BF16).copy(),
        "w2lT": np.asarray(W2l, np.float32).T.astype(BF16).copy(),
        "w2rT": np.asarray(W2r, np.float32).T.astype(BF16).copy(),
        "b1c": np.asarray(b1, np.float32).reshape(D, 1).copy(),
        "b2c": np.asarray(b2, np.float32).reshape(D, 1).copy(),
    }
    in_maps = []
    for c in range(NCORES):
        msgs_pt, dstloc_pt, w_pt, src2_pt, xT = _prep_core(
            c, src, dst, inv, x, K)
        in_maps.append({**common, "msgs": msgs_pt, "dstloc": dstloc_pt,
                        "wts": w_pt, "src2": src2_pt, "xT": xT})

    nc = _build_program(K)
    res = bass_utils.run_bass_kernel_spmd(nc, in_maps, list(range(NCORES)))

    outs = []
    for c in range(NCORES):
        o = res.results[c]["out"]  # [WROWS, 64], row = p*NW + t
        o = o.reshape(P, NW, D).transpose(1, 0, 2).reshape(WROWS, D)[:NLOC]
        outs.append(o)
    return np.concatenate(outs, axis=0).astype(np.float32)



# revision 5
# speedup vs baseline: 1.4932x; 1.4932x over previous
"""Trainium2 Bass kernel for 2-layer GraphSAGE (mean aggregation).

Strategy (8-core SPMD, nodes sharded 12500/core):
- Host: sorts/pads each core's in-edges into fixed 128-edge tiles aligned to
  128-dst-node windows (uniform tile count across cores so one SPMD program
  works), pre-gathers layer-1 messages x[src] (input reindexing) and bakes
  1/deg into per-edge weights.
- Device layer 1: stream pre-gathered messages, segment-sum via one-hot
  indicator matmuls (M[e,r] = (dst_e==r)*w_e built on DVE from an iota tile),
  PSUM-accumulated per 512-node window, then W1l/W1r matmuls + bias + ReLU in
  [feat, node] orientation.
- h1 transposed to row layout via PE-identity matmuls, AllGather -> full
  [100352, 64] bf16 table per core.
- Device layer 2: per-tile indirect-DMA gather of h1 rows, same one-hot
  aggregation, W2l/W2r matmuls + bias, transpose back, DMA out fp32.
"""
import sys

sys.path.insert(0, '/opt/trn_rl_repo')
import numpy as np
import ml_dtypes

BF16 = ml_dtypes.bfloat16
N = 100000
D = 64
NCORES = 8
NLOC = N // NCORES          # 12500
P = 128
NW = (NLOC + P - 1) // P    # 98 dst windows per core
WROWS = NW * P              # 12544 padded local rows
TBL_ROWS = NCORES * WROWS   # 100352 rows in the gathered h1 table


def _layout_row(n):
    """Row index of global node n inside the AllGather'd h1 table."""
    c = n // NLOC
    r = n % NLOC
    t = r // P
    p = r % P
    return c * WROWS + p * NW + t


def _prep_core(c, src, dst, inv, x, K):
    """Slot edges of core c into NW*K tiles of 128, window-aligned."""
    m = (dst >= c * NLOC) & (dst < (c + 1) * NLOC)
    es, ed = src[m], dst[m] - c * NLOC
    w = inv[dst[m]]
    win = ed // P
    order = np.argsort(win, kind='stable')
    es, ed, w, win = es[order], ed[order], w[order], win[order]

    T = NW * K
    slots_src = np.zeros(T * P, dtype=np.int64)
    slots_dstloc = np.full(T * P, -1.0, dtype=np.float32)
    slots_w = np.zeros(T * P, dtype=np.float32)
    # fill window-by-window
    counts = np.bincount(win, minlength=NW)
    starts = np.concatenate([[0], np.cumsum(counts)[:-1]])
    for wi in range(NW):
        cnt = counts[wi]
        base = wi * K * P
        sl = slice(starts[wi], starts[wi] + cnt)
        slots_src[base:base + cnt] = es[sl]
        slots_dstloc[base:base + cnt] = (ed[sl] % P).astype(np.float32)
        slots_w[base:base + cnt] = w[sl]

    # [T*P] slot-major (tile t, partition p = slot t*P+p) -> [P, T] arrays
    def to_pt(a, dt):
        return np.ascontiguousarray(a.reshape(T, P).T.astype(dt))

    dstloc_pt = to_pt(slots_dstloc, np.float32)
    w_pt = to_pt(slots_w, np.float32)
    src2_pt = to_pt(_layout_row(slots_src), np.int32)
    # pre-gathered layer-1 messages, bf16, [P, T*64] partition-major
    msgs = x[slots_src].astype(BF16)           # [T*P, 64]
    msgs_pt = np.ascontiguousarray(
        msgs.reshape(T, P, D).transpose(1, 0, 2).reshape(P, T * D))
    # local x^T padded to WROWS cols
    xT = np.zeros((D, WROWS), dtype=BF16)
    xT[:, :NLOC] = x[c * NLOC:(c + 1) * NLOC].T.astype(BF16)
    return msgs_pt, dstloc_pt, w_pt, src2_pt, xT


def _build_program(K):
    import concourse.bass as bass
    import concourse.tile as tile
    from concourse import bacc, mybir

    T = NW * K
    nc = bacc.Bacc("TRN2", target_bir_lowering=False, debug=False,
                   num_devices=NCORES)
    dt = mybir.dt

    msgs_d = nc.dram_tensor("msgs", [P, T * D], dt.bfloat16, kind="ExternalInput")
    dstloc_d = nc.dram_tensor("dstloc", [P, T], dt.float32, kind="ExternalInput")
    wts_d = nc.dram_tensor("wts", [P, T], dt.float32, kind="ExternalInput")
    src2_d = nc.dram_tensor("src2", [P, T], dt.int32, kind="ExternalInput")
    xT_d = nc.dram_tensor("xT", [D, WROWS], dt.bfloat16, kind="ExternalInput")
    iota_d = nc.dram_tensor("iota", [P, P], dt.bfloat16, kind="ExternalInput")
    id64_d = nc.dram_tensor("id64", [D, D], dt.bfloat16, kind="ExternalInput")
    id64f_d = nc.dram_tensor("id64f", [D, D], dt.float32, kind="ExternalInput")
    w1l_d = nc.dram_tensor("w1lT", [D, D], dt.bfloat16, kind="ExternalInput")
    w1r_d = nc.dram_tensor("w1rT", [D, D], dt.bfloat16, kind="ExternalInput")
    w2l_d = nc.dram_tensor("w2lT", [D, D], dt.bfloat16, kind="ExternalInput")
    w2r_d = nc.dram_tensor("w2rT", [D, D], dt.bfloat16, kind="ExternalInput")
    b1_d = nc.dram_tensor("b1c", [D, 1], dt.float32, kind="ExternalInput")
    b2_d = nc.dram_tensor("b2c", [D, 1], dt.float32, kind="ExternalInput")
    out_d = nc.dram_tensor("out", [WROWS, D], dt.float32, kind="ExternalOutput")

    # supers: groups of up to 4 windows sharing one [64,512] psum bank
    supers = []
    wi = 0
    while wi < NW:
        sw = min(4, NW - wi)
        supers.append((wi, sw))
        wi += sw

    CHUNK_W = 14  # windows of msgs per streamed chunk
    with tile.TileContext(nc) as tc:
        with (
            tc.tile_pool(name="const", bufs=1) as cpool,
            tc.tile_pool(name="chunks", bufs=2) as chpool,
            tc.tile_pool(name="mtiles", bufs=8) as mpool,
            tc.tile_pool(name="gtiles", bufs=12) as gpool,
            tc.tile_pool(name="small", bufs=3) as spool,
            tc.tile_pool(name="psA", bufs=2, space="PSUM") as psA,
            tc.tile_pool(name="psB", bufs=2, space="PSUM") as psB,
            tc.tile_pool(name="psT", bufs=2, space="PSUM") as psT,
            tc.tile_pool(name="dram", bufs=1, space="DRAM") as dpool,
        ):
            # resident SBUF state
            dstloc_sb = cpool.tile([P, T], dt.float32, tag="dstloc")
            wts_sb = cpool.tile([P, T], dt.float32, tag="wts")
            src2_sb = cpool.tile([P, T], dt.int32, tag="src2")
            xT_sb = cpool.tile([D, WROWS], dt.bfloat16, tag="xT")
            iota_sb = cpool.tile([P, P], dt.bfloat16, tag="iota")
            id64_sb = cpool.tile([D, D], dt.bfloat16, tag="id64")
            id64f_sb = cpool.tile([D, D], dt.float32, tag="id64f")
            w1l_sb = cpool.tile([D, D], dt.bfloat16, tag="w1l")
            w1r_sb = cpool.tile([D, D], dt.bfloat16, tag="w1r")
            w2l_sb = cpool.tile([D, D], dt.bfloat16, tag="w2l")
            w2r_sb = cpool.tile([D, D], dt.bfloat16, tag="w2r")
            b1_sb = cpool.tile([D, 1], dt.float32, tag="b1")
            b2_sb = cpool.tile([D, 1], dt.float32, tag="b2")
            h1T_sb = cpool.tile([D, WROWS], dt.bfloat16, tag="h1T")
            h1rows_sb = cpool.tile([P, NW * D], dt.bfloat16, tag="h1rows")

            for t_sb, t_d in [(dstloc_sb, dstloc_d), (wts_sb, wts_d),
                              (src2_sb, src2_d), (xT_sb, xT_d),
                              (iota_sb, iota_d), (id64_sb, id64_d),
                              (id64f_sb, id64f_d),
                              (w1l_sb, w1l_d), (w1r_sb, w1r_d),
                              (w2l_sb, w2l_d), (w2r_sb, w2r_d),
                              (b1_sb, b1_d), (b2_sb, b2_d)]:
                nc.sync.dma_start(out=t_sb[:], in_=t_d.ap())

            h1loc_dram = dpool.tile([WROWS, D], dt.bfloat16, tag="h1loc")
            h1full_dram = dpool.tile([TBL_ROWS, D], dt.bfloat16, tag="h1full")

            # ---------------- layer 1 ----------------
            nchunks = (NW + CHUNK_W - 1) // CHUNK_W
            chunk_tiles = {}
            for ci in range(nchunks):
                w0 = ci * CHUNK_W
                nw = min(CHUNK_W, NW - w0)
                ch = chpool.tile([P, CHUNK_W * K * D], dt.bfloat16, tag="msgs")
                nc.sync.dma_start(
                    out=ch[:, :nw * K * D],
                    in_=msgs_d.ap()[:, w0 * K * D:(w0 + nw) * K * D])
                chunk_tiles[ci] = ch

            for w0, sw in supers:
                agg_ps = psA.tile([D, 4 * P], dt.float32, tag="agg")
                for s in range(sw):
                    wi = w0 + s
                    ci, woff = wi // CHUNK_W, wi % CHUNK_W
                    ch = chunk_tiles[ci]
                    for k in range(K):
                        t = wi * K + k
                        mt = mpool.tile([P, P], dt.bfloat16, tag="M")
                        nc.vector.tensor_scalar(
                            out=mt[:], in0=iota_sb[:],
                            scalar1=dstloc_sb[:, t:t + 1],
                            scalar2=wts_sb[:, t:t + 1],
                            op0=mybir.AluOpType.is_equal,
                            op1=mybir.AluOpType.mult)
                        nc.tensor.matmul(
                            out=agg_ps[:, s * P:(s + 1) * P],
                            lhsT=ch[:, (woff * K + k) * D:(woff * K + k + 1) * D],
                            rhs=mt[:], start=(k == 0), stop=(k == K - 1))
                agg_sb = spool.tile([D, 4 * P], dt.bfloat16, tag="aggsb")
                nc.vector.tensor_copy(out=agg_sb[:, :sw * P],
                                      in_=agg_ps[:, :sw * P])
                h_ps = psB.tile([D, 4 * P], dt.float32, tag="hps")
                nc.tensor.matmul(out=h_ps[:, :sw * P], lhsT=w1l_sb[:],
                                 rhs=agg_sb[:, :sw * P], start=True, stop=False)
                nc.tensor.matmul(out=h_ps[:, :sw * P], lhsT=w1r_sb[:],
                                 rhs=xT_sb[:, w0 * P:(w0 + sw) * P],
                                 start=False, stop=True)
                nc.scalar.activation(
                    out=h1T_sb[:, w0 * P:(w0 + sw) * P], in_=h_ps[:, :sw * P],
                    func=mybir.ActivationFunctionType.Relu, bias=b1_sb[:])

            # transpose h1T -> rows, stage, one DMA to DRAM, AllGather
            for wi in range(NW):
                tp = psT.tile([P, D], dt.float32, tag="tp")
                nc.tensor.matmul(out=tp[:], lhsT=h1T_sb[:, wi * P:(wi + 1) * P],
                                 rhs=id64_sb[:], start=True, stop=True)
                nc.vector.tensor_copy(out=h1rows_sb[:, wi * D:(wi + 1) * D],
                                      in_=tp[:])
            nc.sync.dma_start(
                out=h1loc_dram[:].rearrange("(p t) f -> p (t f)", p=P),
                in_=h1rows_sb[:])
            nc.gpsimd.collective_compute(
                "AllGather", mybir.AluOpType.bypass,
                replica_groups=[list(range(NCORES))],
                ins=[h1loc_dram[:]], outs=[h1full_dram[:]])

            # ---------------- layer 2 ----------------
            out2T_sb = cpool.tile([D, WROWS], dt.float32, tag="out2T")
            for w0, sw in supers:
                agg_ps = psA.tile([D, 4 * P], dt.float32, tag="agg")
                for s in range(sw):
                    wi = w0 + s
                    for k in range(K):
                        t = wi * K + k
                        gt = gpool.tile([P, D], dt.bfloat16, tag="g")
                        nc.gpsimd.indirect_dma_start(
                            out=gt[:], out_offset=None, in_=h1full_dram[:],
                            in_offset=bass.IndirectOffsetOnAxis(
                                ap=src2_sb[:, t:t + 1], axis=0))
                        mt = mpool.tile([P, P], dt.bfloat16, tag="M")
                        nc.vector.tensor_scalar(
                            out=mt[:], in0=iota_sb[:],
                            scalar1=dstloc_sb[:, t:t + 1],
                            scalar2=wts_sb[:, t:t + 1],
                            op0=mybir.AluOpType.is_equal,
                            op1=mybir.AluOpType.mult)
                        nc.tensor.matmul(
                            out=agg_ps[:, s * P:(s + 1) * P], lhsT=gt[:],
                            rhs=mt[:], start=(k == 0), stop=(k == K - 1))
                agg_sb = spool.tile([D, 4 * P], dt.bfloat16, tag="aggsb")
                nc.vector.tensor_copy(out=agg_sb[:, :sw * P],
                                      in_=agg_ps[:, :sw * P])
                h_ps = psB.tile([D, 4 * P], dt.float32, tag="hps")
                nc.tensor.matmul(out=h_ps[:, :sw * P], lhsT=w2l_sb[:],
                                 rhs=agg_sb[:, :sw * P], start=True, stop=False)
                nc.tensor.matmul(out=h_ps[:, :sw * P], lhsT=w2r_sb[:],
                                 rhs=h1T_sb[:, w0 * P:(w0 + sw) * P],
                                 start=False, stop=True)
                nc.vector.tensor_scalar_add(
                    out=out2T_sb[:, w0 * P:(w0 + sw) * P],
                    in0=h_ps[:, :sw * P], scalar1=b2_sb[:])

            # transpose out2T -> rows -> DRAM
            outv = out_d.ap().rearrange("(p t) f -> p t f", p=P)
            for wi in range(NW):
                tp = psT.tile([P, D], dt.float32, tag="tp")
                nc.tensor.matmul(out=tp[:], lhsT=out2T_sb[:, wi * P:(wi + 1) * P],
                                 rhs=id64f_sb[:], start=True, stop=True)
                ot = spool.tile([P, D], dt.float32, tag="orow")
                nc.vector.tensor_copy(out=ot[:], in_=tp[:])
                nc.sync.dma_start(out=outv[:, wi, :], in_=ot[:])

    nc.compile()
    return nc


def kernel(x, edge_index, W1l, W1r, b1, W2l, W2r, b2):
    from concourse import bass_utils

    x = np.asarray(x, dtype=np.float32)
    edge_index = np.asarray(edge_index)
    src = edge_index[0].astype(np.int64)
    dst = edge_index[1].astype(np.int64)
    cnt = np.bincount(dst, minlength=N).astype(np.float32)
    inv = (1.0 / np.maximum(cnt, 1.0)).astype(np.float32)

    # uniform tile count across cores/windows
    wid = (dst % NLOC) // P + (dst // NLOC) * NW
    wc = np.bincount(wid, minlength=NCORES * NW)
    K = int(np.max((wc + P - 1) // P))
    K = max(K, 1)

    iota = np.tile(np.arange(P, dtype=np.float32), (P, 1)).astype(BF16)
    id64 = np.eye(D, dtype=np.float32)
    common = {
        "iota": iota, "id64": id64.astype(BF16), "id64f": id64,
        "w1lT": np.asarray(W1l, np.float32).T.astype(BF16).copy(),
        "w1rT": np.asarray(W1r, np.float32).T.astype(BF16).copy(),
        "w2lT": np.asarray(W2l, np.float32).T.astype(BF16).copy(),
        "w2rT": np.asarray(W2r, np.float32).T.astype(BF16).copy(),
        "b1c": np.asarray(b1, np.float32).reshape(D, 1).copy(),
        "b2c": np.asarray(b2, np.float32).reshape(D, 1).copy(),
    }
    in_maps = []
    for c in range(NCORES):
        msgs_pt, dstloc_pt, w_pt, src2_pt, xT = _prep_core(
            c, src, dst, inv, x, K)
        in_maps.append({**common, "msgs": msgs_pt, "dstloc": dstloc_pt,
                        "wts": w_pt, "src2": src2_pt, "xT": xT})

    nc = _build_program(K)
    res = bass_utils.run_bass_kernel_spmd(nc, in_maps, list(range(NCORES)))

    outs = []
    for c in range(NCORES):
        o = res.results[c]["out"]  # [WROWS, 64], row = p*NW + t
        o = o.reshape(P, NW, D).transpose(1, 0, 2).reshape(WROWS, D)[:NLOC]
        outs.append(o)
    return np.concatenate(outs, axis=0).astype(np.float32)



# revision 9
# speedup vs baseline: 1.5480x; 1.0367x over previous
"""Trainium2 Bass kernel for 2-layer GraphSAGE (mean aggregation).

Strategy (8-core SPMD, nodes sharded 12500/core):
- Host: sorts/pads each core's in-edges into fixed 128-edge tiles aligned to
  128-dst-node windows (uniform tile count across cores so one SPMD program
  works), pre-gathers layer-1 messages x[src] (input reindexing) and bakes
  1/deg into per-edge weights.
- Device layer 1: stream pre-gathered messages, segment-sum via one-hot
  indicator matmuls (M[e,r] = (dst_e==r)*w_e built on DVE from an iota tile),
  PSUM-accumulated per 512-node window, then W1l/W1r matmuls + bias + ReLU in
  [feat, node] orientation.
- h1 transposed to row layout via PE-identity matmuls, AllGather -> full
  [100352, 64] bf16 table per core.
- Device layer 2: per-tile indirect-DMA gather of h1 rows, same one-hot
  aggregation, W2l/W2r matmuls + bias, transpose back, DMA out fp32.
"""
import sys

sys.path.insert(0, '/opt/trn_rl_repo')
import numpy as np
import ml_dtypes

BF16 = ml_dtypes.bfloat16
N = 100000
D = 64
NCORES = 8
NLOC = N // NCORES          # 12500
P = 128
NW = (NLOC + P - 1) // P    # 98 dst windows per core
WROWS = NW * P              # 12544 padded local rows
TBL_ROWS = NCORES * WROWS   # 100352 rows in the gathered h1 table


AG_W = 14                   # windows per AllGather chunk (7 chunks)
AG_ROWS = P * AG_W          # 1792 rows per core per chunk


def _layout_row(n):
    """Row index of global node n inside the AllGather'd h1 table.

    Chunk-major so each 14-window chunk AllGathers into a contiguous
    [8*AG_ROWS, D] slice: row = g*8*AG_ROWS + c*AG_ROWS + p*AG_W + t%AG_W.
    """
    c = n // NLOC
    r = n % NLOC
    t = r // P
    p = r % P
    g = t // AG_W
    return g * (NCORES * AG_ROWS) + c * AG_ROWS + p * AG_W + (t % AG_W)


def _prep_core(c, src, dst, inv, x, K):
    """Slot edges of core c into NW*K tiles of 128, window-aligned."""
    m = (dst >= c * NLOC) & (dst < (c + 1) * NLOC)
    es, ed = src[m], dst[m] - c * NLOC
    w = inv[dst[m]]
    win = ed // P
    order = np.argsort(win, kind='stable')
    es, ed, w, win = es[order], ed[order], w[order], win[order]

    T = NW * K
    slots_src = np.zeros(T * P, dtype=np.int64)
    slots_dstloc = np.full(T * P, -1.0, dtype=np.float32)
    slots_w = np.zeros(T * P, dtype=np.float32)
    # fill window-by-window
    counts = np.bincount(win, minlength=NW)
    starts = np.concatenate([[0], np.cumsum(counts)[:-1]])
    for wi in range(NW):
        cnt = counts[wi]
        base = wi * K * P
        sl = slice(starts[wi], starts[wi] + cnt)
        slots_src[base:base + cnt] = es[sl]
        slots_dstloc[base:base + cnt] = (ed[sl] % P).astype(np.float32)
        slots_w[base:base + cnt] = w[sl]

    # [T*P] slot-major (tile t, partition p = slot t*P+p) -> [P, T] arrays
    def to_pt(a, dt):
        return np.ascontiguousarray(a.reshape(T, P).T.astype(dt))

    dstloc_pt = to_pt(slots_dstloc, np.float32)
    w_pt = to_pt(slots_w, np.float32)
    src2_pt = to_pt(_layout_row(slots_src), np.int32)
    # pre-gathered layer-1 messages, bf16, [P, T*64] partition-major
    msgs = x[slots_src].astype(BF16)           # [T*P, 64]
    msgs_pt = np.ascontiguousarray(
        msgs.reshape(T, P, D).transpose(1, 0, 2).reshape(P, T * D))
    # local x^T padded to WROWS cols
    xT = np.zeros((D, WROWS), dtype=BF16)
    xT[:, :NLOC] = x[c * NLOC:(c + 1) * NLOC].T.astype(BF16)
    return msgs_pt, dstloc_pt, w_pt, src2_pt, xT


def _build_program(K):
    import concourse.bass as bass
    import concourse.tile as tile
    from concourse import bacc, mybir

    T = NW * K
    nc = bacc.Bacc("TRN2", target_bir_lowering=False, debug=False,
                   num_devices=NCORES)
    dt = mybir.dt

    msgs_d = nc.dram_tensor("msgs", [P, T * D], dt.bfloat16, kind="ExternalInput")
    dstloc_d = nc.dram_tensor("dstloc", [P, T], dt.float32, kind="ExternalInput")
    wts_d = nc.dram_tensor("wts", [P, T], dt.float32, kind="ExternalInput")
    src2_d = nc.dram_tensor("src2", [P, T], dt.int32, kind="ExternalInput")
    xT_d = nc.dram_tensor("xT", [D, WROWS], dt.bfloat16, kind="ExternalInput")
    iota_d = nc.dram_tensor("iota", [P, P], dt.bfloat16, kind="ExternalInput")
    id64_d = nc.dram_tensor("id64", [D, D], dt.bfloat16, kind="ExternalInput")
    id64f_d = nc.dram_tensor("id64f", [D, D], dt.float32, kind="ExternalInput")
    w1l_d = nc.dram_tensor("w1lT", [D, D], dt.bfloat16, kind="ExternalInput")
    w1r_d = nc.dram_tensor("w1rT", [D, D], dt.bfloat16, kind="ExternalInput")
    w2l_d = nc.dram_tensor("w2lT", [D, D], dt.bfloat16, kind="ExternalInput")
    w2r_d = nc.dram_tensor("w2rT", [D, D], dt.bfloat16, kind="ExternalInput")
    b1_d = nc.dram_tensor("b1c", [D, 1], dt.float32, kind="ExternalInput")
    b2_d = nc.dram_tensor("b2c", [D, 1], dt.float32, kind="ExternalInput")
    out_d = nc.dram_tensor("out", [WROWS, D], dt.float32, kind="ExternalOutput")

    # supers: groups of 2 windows sharing one psum bank (7 supers = 1 AG chunk)
    supers = []
    wi = 0
    while wi < NW:
        sw = min(2, NW - wi)
        supers.append((wi, sw))
        wi += sw

    CHUNK_W = 14  # windows of msgs per streamed chunk
    with tile.TileContext(nc) as tc:
        with (
            tc.tile_pool(name="const", bufs=1) as cpool,
            tc.tile_pool(name="chunks", bufs=2) as chpool,
            tc.tile_pool(name="mtiles", bufs=12) as mpool,
            tc.tile_pool(name="gtiles", bufs=20) as gpool,
            tc.tile_pool(name="small", bufs=3) as spool,
            tc.tile_pool(name="psA", bufs=2, space="PSUM") as psA,
            tc.tile_pool(name="psB", bufs=2, space="PSUM") as psB,
            tc.tile_pool(name="psT", bufs=2, space="PSUM") as psT,
            tc.tile_pool(name="dram", bufs=1, space="DRAM") as dpool,
        ):
            # resident SBUF state
            dstloc_sb = cpool.tile([P, T], dt.float32, tag="dstloc")
            wts_sb = cpool.tile([P, T], dt.float32, tag="wts")
            src2_sb = cpool.tile([P, T], dt.int32, tag="src2")
            xT_sb = cpool.tile([D, WROWS], dt.bfloat16, tag="xT")
            iota_sb = cpool.tile([P, P], dt.bfloat16, tag="iota")
            id64_sb = cpool.tile([D, D], dt.bfloat16, tag="id64")
            id64f_sb = cpool.tile([D, D], dt.float32, tag="id64f")
            w1l_sb = cpool.tile([D, D], dt.bfloat16, tag="w1l")
            w1r_sb = cpool.tile([D, D], dt.bfloat16, tag="w1r")
            w2l_sb = cpool.tile([D, D], dt.bfloat16, tag="w2l")
            w2r_sb = cpool.tile([D, D], dt.bfloat16, tag="w2r")
            b1_sb = cpool.tile([D, 1], dt.float32, tag="b1")
            b2_sb = cpool.tile([D, 1], dt.float32, tag="b2")
            h1T_sb = cpool.tile([D, WROWS], dt.bfloat16, tag="h1T")
            h1rows_sb = cpool.tile([P, NW * D], dt.bfloat16, tag="h1rows")

            for t_sb, t_d in [(dstloc_sb, dstloc_d), (wts_sb, wts_d),
                              (src2_sb, src2_d), (xT_sb, xT_d),
                              (iota_sb, iota_d), (id64_sb, id64_d),
                              (id64f_sb, id64f_d),
                              (w1l_sb, w1l_d), (w1r_sb, w1r_d),
                              (w2l_sb, w2l_d), (w2r_sb, w2r_d),
                              (b1_sb, b1_d), (b2_sb, b2_d)]:
                nc.sync.dma_start(out=t_sb[:], in_=t_d.ap())

            h1loc_dram = dpool.tile([WROWS, D], dt.bfloat16, tag="h1loc")
            h1full_dram = dpool.tile([TBL_ROWS, D], dt.bfloat16, tag="h1full")

            # ---------------- layer 1 ----------------
            nchunks = (NW + CHUNK_W - 1) // CHUNK_W
            chunk_tiles = {}
            for ci in range(nchunks):
                w0 = ci * CHUNK_W
                nw = min(CHUNK_W, NW - w0)
                ch = chpool.tile([P, CHUNK_W * K * D], dt.bfloat16, tag="msgs")
                nc.sync.dma_start(
                    out=ch[:, :nw * K * D],
                    in_=msgs_d.ap()[:, w0 * K * D:(w0 + nw) * K * D])
                chunk_tiles[ci] = ch

            for w0, sw in supers:
                agg_ps = psA.tile([D, 4 * P], dt.float32, tag="agg")
                for s in range(sw):
                    wi = w0 + s
                    ci, woff = wi // CHUNK_W, wi % CHUNK_W
                    ch = chunk_tiles[ci]
                    for k in range(K):
                        t = wi * K + k
                        mt = mpool.tile([P, P], dt.bfloat16, tag="M")
                        nc.vector.tensor_scalar(
                            out=mt[:], in0=iota_sb[:],
                            scalar1=dstloc_sb[:, t:t + 1],
                            scalar2=wts_sb[:, t:t + 1],
                            op0=mybir.AluOpType.is_equal,
                            op1=mybir.AluOpType.mult)
                        nc.tensor.matmul(
                            out=agg_ps[:, s * P:(s + 1) * P],
                            lhsT=ch[:, (woff * K + k) * D:(woff * K + k + 1) * D],
                            rhs=mt[:], start=(k == 0), stop=(k == K - 1))
                agg_sb = spool.tile([D, 4 * P], dt.bfloat16, tag="aggsb")
                nc.vector.tensor_copy(out=agg_sb[:, :sw * P],
                                      in_=agg_ps[:, :sw * P])
                h_ps = psB.tile([D, 4 * P], dt.float32, tag="hps")
                nc.tensor.matmul(out=h_ps[:, :sw * P], lhsT=w1l_sb[:],
                                 rhs=agg_sb[:, :sw * P], start=True, stop=False)
                nc.tensor.matmul(out=h_ps[:, :sw * P], lhsT=w1r_sb[:],
                                 rhs=xT_sb[:, w0 * P:(w0 + sw) * P],
                                 start=False, stop=True)
                nc.scalar.activation(
                    out=h1T_sb[:, w0 * P:(w0 + sw) * P], in_=h_ps[:, :sw * P],
                    func=mybir.ActivationFunctionType.Relu, bias=b1_sb[:])
                # transpose this super's windows into row staging
                for wi in range(w0, w0 + sw):
                    tp = psT.tile([P, D], dt.float32, tag="tp")
                    nc.tensor.matmul(out=tp[:],
                                     lhsT=h1T_sb[:, wi * P:(wi + 1) * P],
                                     rhs=id64_sb[:], start=True, stop=True)
                    nc.vector.tensor_copy(out=h1rows_sb[:, wi * D:(wi + 1) * D],
                                          in_=tp[:])
                # chunk complete -> stage to DRAM + AllGather (overlaps L1)
                if (w0 + sw) % AG_W == 0:
                    g = (w0 + sw) // AG_W - 1
                    nc.sync.dma_start(
                        out=h1loc_dram[g * AG_ROWS:(g + 1) * AG_ROWS]
                        .rearrange("(p t) f -> p (t f)", p=P),
                        in_=h1rows_sb[:, g * AG_W * D:(g + 1) * AG_W * D])
                    nc.gpsimd.collective_compute(
                        "AllGather", mybir.AluOpType.bypass,
                        replica_groups=[list(range(NCORES))],
                        ins=[h1loc_dram[g * AG_ROWS:(g + 1) * AG_ROWS]],
                        outs=[h1full_dram[g * NCORES * AG_ROWS:
                                          (g + 1) * NCORES * AG_ROWS]])

            # ---------------- layer 2 ----------------
            out2T_sb = cpool.tile([D, WROWS], dt.float32, tag="out2T")
            for w0, sw in supers:
                agg_ps = psA.tile([D, 4 * P], dt.float32, tag="agg")
                for s in range(sw):
                    wi = w0 + s
                    for k in range(K):
                        t = wi * K + k
                        gt = gpool.tile([P, D], dt.bfloat16, tag="g")
                        nc.gpsimd.indirect_dma_start(
                            out=gt[:], out_offset=None, in_=h1full_dram[:],
                            in_offset=bass.IndirectOffsetOnAxis(
                                ap=src2_sb[:, t:t + 1], axis=0))
                        mt = mpool.tile([P, P], dt.bfloat16, tag="M")
                        nc.vector.tensor_scalar(
                            out=mt[:], in0=iota_sb[:],
                            scalar1=dstloc_sb[:, t:t + 1],
                            scalar2=wts_sb[:, t:t + 1],
                            op0=mybir.AluOpType.is_equal,
                            op1=mybir.AluOpType.mult)
                        nc.tensor.matmul(
                            out=agg_ps[:, s * P:(s + 1) * P], lhsT=gt[:],
                            rhs=mt[:], start=(k == 0), stop=(k == K - 1))
                agg_sb = spool.tile([D, 4 * P], dt.bfloat16, tag="aggsb")
                nc.vector.tensor_copy(out=agg_sb[:, :sw * P],
                                      in_=agg_ps[:, :sw * P])
                h_ps = psB.tile([D, 4 * P], dt.float32, tag="hps")
                nc.tensor.matmul(out=h_ps[:, :sw * P], lhsT=w2l_sb[:],
                                 rhs=agg_sb[:, :sw * P], start=True, stop=False)
                nc.tensor.matmul(out=h_ps[:, :sw * P], lhsT=w2r_sb[:],
                                 rhs=h1T_sb[:, w0 * P:(w0 + sw) * P],
                                 start=False, stop=True)
                nc.vector.tensor_scalar_add(
                    out=out2T_sb[:, w0 * P:(w0 + sw) * P],
                    in0=h_ps[:, :sw * P], scalar1=b2_sb[:])

            # transpose out2T -> rows -> DRAM
            outv = out_d.ap().rearrange("(p t) f -> p t f", p=P)
            for wi in range(NW):
                tp = psT.tile([P, D], dt.float32, tag="tp")
                nc.tensor.matmul(out=tp[:], lhsT=out2T_sb[:, wi * P:(wi + 1) * P],
                                 rhs=id64f_sb[:], start=True, stop=True)
                ot = spool.tile([P, D], dt.float32, tag="orow")
                nc.vector.tensor_copy(out=ot[:], in_=tp[:])
                nc.sync.dma_start(out=outv[:, wi, :], in_=ot[:])

    nc.compile()
    return nc


def kernel(x, edge_index, W1l, W1r, b1, W2l, W2r, b2):
    from concourse import bass_utils

    x = np.asarray(x, dtype=np.float32)
    edge_index = np.asarray(edge_index)
    src = edge_index[0].astype(np.int64)
    dst = edge_index[1].astype(np.int64)
    cnt = np.bincount(dst, minlength=N).astype(np.float32)
    inv = (1.0 / np.maximum(cnt, 1.0)).astype(np.float32)

    # uniform tile count across cores/windows
    wid = (dst % NLOC) // P + (dst // NLOC) * NW
    wc = np.bincount(wid, minlength=NCORES * NW)
    K = int(np.max((wc + P - 1) // P))
    K = max(K, 1)

    iota = np.tile(np.arange(P, dtype=np.float32), (P, 1)).astype(BF16)
    id64 = np.eye(D, dtype=np.float32)
    common = {
        "iota": iota, "id64": id64.astype(BF16), "id64f": id64,
        "w1lT": np.asarray(W1l, np.float32).T.astype(BF16).copy(),
        "w1rT": np.asarray(W1r, np.float32).T.astype(BF16).copy(),
        "w2lT": np.asarray(W2l, np.float32).T.astype(BF16).copy(),
        "w2rT": np.asarray(W2r, np.float32).T.astype(BF16).copy(),
        "b1c": np.asarray(b1, np.float32).reshape(D, 1).copy(),
        "b2c": np.asarray(b2, np.float32).reshape(D, 1).copy(),
    }
    in_maps = []
    for c in range(NCORES):
        msgs_pt, dstloc_pt, w_pt, src2_pt, xT = _prep_core(
            c, src, dst, inv, x, K)
        in_maps.append({**common, "msgs": msgs_pt, "dstloc": dstloc_pt,
                        "wts": w_pt, "src2": src2_pt, "xT": xT})

    nc = _build_program(K)
    res = bass_utils.run_bass_kernel_spmd(nc, in_maps, list(range(NCORES)))

    outs = []
    for c in range(NCORES):
        o = res.results[c]["out"]  # [WROWS, 64], row = p*NW + t
        o = o.reshape(P, NW, D).transpose(1, 0, 2).reshape(WROWS, D)[:NLOC]
        outs.append(o)
    return np.concatenate(outs, axis=0).astype(np.float32)



# revision 12
# speedup vs baseline: 1.6279x; 1.0516x over previous
"""Trainium2 Bass kernel for 2-layer GraphSAGE (mean aggregation).

Strategy (8-core SPMD, nodes sharded 12500/core):
- Host: sorts/pads each core's in-edges into fixed 128-edge tiles aligned to
  128-dst-node windows (uniform tile count across cores so one SPMD program
  works), pre-gathers layer-1 messages x[src] (input reindexing) and bakes
  1/deg into per-edge weights.
- Device layer 1: stream pre-gathered messages, segment-sum via one-hot
  indicator matmuls (M[e,r] = (dst_e==r)*w_e built on DVE from an iota tile),
  PSUM-accumulated per 512-node window, then W1l/W1r matmuls + bias + ReLU in
  [feat, node] orientation.
- h1 transposed to row layout via PE-identity matmuls, AllGather -> full
  [100352, 64] bf16 table per core.
- Device layer 2: per-tile indirect-DMA gather of h1 rows, same one-hot
  aggregation, W2l/W2r matmuls + bias, transpose back, DMA out fp32.
"""
import sys

sys.path.insert(0, '/opt/trn_rl_repo')
import numpy as np
import ml_dtypes

BF16 = ml_dtypes.bfloat16
N = 100000
D = 64
NCORES = 8
NLOC = N // NCORES          # 12500
P = 128
NW = (NLOC + P - 1) // P    # 98 dst windows per core
WROWS = NW * P              # 12544 padded local rows
TBL_ROWS = NCORES * WROWS   # 100352 rows in the gathered h1 table


AG_W = 14                   # windows per AllGather chunk (7 chunks)
AG_ROWS = P * AG_W          # 1792 rows per core per chunk


def _layout_row(n):
    """Row index of global node n inside the AllGather'd h1 table.

    Chunk-major so each 14-window chunk AllGathers into a contiguous
    [8*AG_ROWS, D] slice: row = g*8*AG_ROWS + c*AG_ROWS + p*AG_W + t%AG_W.
    """
    c = n // NLOC
    r = n % NLOC
    t = r // P
    p = r % P
    g = t // AG_W
    return g * (NCORES * AG_ROWS) + c * AG_ROWS + p * AG_W + (t % AG_W)


def _prep_core(c, src, dst, inv, x, Kw, offw):
    """Slot edges of core c into per-window Kw[wi] tiles of 128."""
    m = (dst >= c * NLOC) & (dst < (c + 1) * NLOC)
    es, ed = src[m], dst[m] - c * NLOC
    w = inv[dst[m]]
    win = ed // P
    order = np.argsort(win, kind='stable')
    es, ed, w, win = es[order], ed[order], w[order], win[order]

    T = int(offw[-1])
    slots_src = np.zeros(T * P, dtype=np.int64)
    slots_dstloc = np.full(T * P, -1.0, dtype=np.float32)
    slots_w = np.zeros(T * P, dtype=np.float32)
    # fill window-by-window
    counts = np.bincount(win, minlength=NW)
    starts = np.concatenate([[0], np.cumsum(counts)[:-1]])
    for wi in range(NW):
        cnt = counts[wi]
        base = int(offw[wi]) * P
        sl = slice(starts[wi], starts[wi] + cnt)
        slots_src[base:base + cnt] = es[sl]
        slots_dstloc[base:base + cnt] = (ed[sl] % P).astype(np.float32)
        slots_w[base:base + cnt] = w[sl]

    # [T*P] slot-major (tile t, partition p = slot t*P+p) -> [P, T] arrays
    def to_pt(a, dt):
        return np.ascontiguousarray(a.reshape(T, P).T.astype(dt))

    dstloc_pt = to_pt(slots_dstloc, np.float32)
    w_pt = to_pt(slots_w, np.float32)
    src2_pt = to_pt(_layout_row(slots_src), np.int32)
    # pre-gathered layer-1 messages, bf16, [P, T*64] partition-major
    msgs = x[slots_src].astype(BF16)           # [T*P, 64]
    msgs_pt = np.ascontiguousarray(
        msgs.reshape(T, P, D).transpose(1, 0, 2).reshape(P, T * D))
    # local x^T padded to WROWS cols
    xT = np.zeros((D, WROWS), dtype=BF16)
    xT[:, :NLOC] = x[c * NLOC:(c + 1) * NLOC].T.astype(BF16)
    return msgs_pt, dstloc_pt, w_pt, src2_pt, xT


def _build_program(Kw, offw):
    import concourse.bass as bass
    import concourse.tile as tile
    from concourse import bacc, mybir

    T = int(offw[-1])
    nc = bacc.Bacc("TRN2", target_bir_lowering=False, debug=False,
                   num_devices=NCORES)
    dt = mybir.dt

    msgs_d = nc.dram_tensor("msgs", [P, T * D], dt.bfloat16, kind="ExternalInput")
    dstloc_d = nc.dram_tensor("dstloc", [P, T], dt.float32, kind="ExternalInput")
    wts_d = nc.dram_tensor("wts", [P, T], dt.float32, kind="ExternalInput")
    src2_d = nc.dram_tensor("src2", [P, T], dt.int32, kind="ExternalInput")
    xT_d = nc.dram_tensor("xT", [D, WROWS], dt.bfloat16, kind="ExternalInput")
    iota_d = nc.dram_tensor("iota", [P, P], dt.bfloat16, kind="ExternalInput")
    id64_d = nc.dram_tensor("id64", [D, D], dt.bfloat16, kind="ExternalInput")
    id64f_d = nc.dram_tensor("id64f", [D, D], dt.float32, kind="ExternalInput")
    w1l_d = nc.dram_tensor("w1lT", [D, D], dt.bfloat16, kind="ExternalInput")
    w1r_d = nc.dram_tensor("w1rT", [D, D], dt.bfloat16, kind="ExternalInput")
    w2l_d = nc.dram_tensor("w2lT", [D, D], dt.bfloat16, kind="ExternalInput")
    w2r_d = nc.dram_tensor("w2rT", [D, D], dt.bfloat16, kind="ExternalInput")
    b1_d = nc.dram_tensor("b1c", [D, 1], dt.float32, kind="ExternalInput")
    b2_d = nc.dram_tensor("b2c", [D, 1], dt.float32, kind="ExternalInput")
    out_d = nc.dram_tensor("out", [WROWS, D], dt.float32, kind="ExternalOutput")

    # supers: groups of 2 windows sharing one psum bank (7 supers = 1 AG chunk)
    supers = []
    wi = 0
    while wi < NW:
        sw = min(2, NW - wi)
        supers.append((wi, sw))
        wi += sw

    CHUNK_W = 14  # windows of msgs per streamed chunk
    with tile.TileContext(nc) as tc:
        with (
            tc.tile_pool(name="const", bufs=1) as cpool,
            tc.tile_pool(name="chunks", bufs=2) as chpool,
            tc.tile_pool(name="mtiles", bufs=12) as mpool,
            tc.tile_pool(name="gtiles", bufs=20) as gpool,
            tc.tile_pool(name="small", bufs=3) as spool,
            tc.tile_pool(name="psA", bufs=2, space="PSUM") as psA,
            tc.tile_pool(name="psB", bufs=2, space="PSUM") as psB,
            tc.tile_pool(name="psT", bufs=2, space="PSUM") as psT,
            tc.tile_pool(name="dram", bufs=1, space="DRAM") as dpool,
        ):
            # resident SBUF state
            dstloc_sb = cpool.tile([P, T], dt.float32, tag="dstloc")
            wts_sb = cpool.tile([P, T], dt.float32, tag="wts")
            src2_sb = cpool.tile([P, T], dt.int32, tag="src2")
            xT_sb = cpool.tile([D, WROWS], dt.bfloat16, tag="xT")
            iota_sb = cpool.tile([P, P], dt.bfloat16, tag="iota")
            id64_sb = cpool.tile([D, D], dt.bfloat16, tag="id64")
            id64f_sb = cpool.tile([D, D], dt.float32, tag="id64f")
            w1l_sb = cpool.tile([D, D], dt.bfloat16, tag="w1l")
            w1r_sb = cpool.tile([D, D], dt.bfloat16, tag="w1r")
            w2l_sb = cpool.tile([D, D], dt.bfloat16, tag="w2l")
            w2r_sb = cpool.tile([D, D], dt.bfloat16, tag="w2r")
            b1_sb = cpool.tile([D, 1], dt.float32, tag="b1")
            b2_sb = cpool.tile([D, 1], dt.float32, tag="b2")
            h1T_sb = cpool.tile([D, WROWS], dt.bfloat16, tag="h1T")
            h1rows_sb = cpool.tile([P, NW * D], dt.bfloat16, tag="h1rows")

            for t_sb, t_d in [(dstloc_sb, dstloc_d), (wts_sb, wts_d),
                              (src2_sb, src2_d), (xT_sb, xT_d),
                              (iota_sb, iota_d), (id64_sb, id64_d),
                              (id64f_sb, id64f_d),
                              (w1l_sb, w1l_d), (w1r_sb, w1r_d),
                              (w2l_sb, w2l_d), (w2r_sb, w2r_d),
                              (b1_sb, b1_d), (b2_sb, b2_d)]:
                nc.sync.dma_start(out=t_sb[:], in_=t_d.ap())

            h1loc_dram = dpool.tile([WROWS, D], dt.bfloat16, tag="h1loc")
            h1full_dram = dpool.tile([TBL_ROWS, D], dt.bfloat16, tag="h1full")

            # ---------------- layer 1 ----------------
            nchunks = (NW + CHUNK_W - 1) // CHUNK_W
            maxc = max(int(offw[min(ci * CHUNK_W + CHUNK_W, NW)]
                           - offw[ci * CHUNK_W]) for ci in range(nchunks))
            chunk_tiles = {}
            for ci in range(nchunks):
                w0 = ci * CHUNK_W
                nw = min(CHUNK_W, NW - w0)
                c0, c1 = int(offw[w0]), int(offw[w0 + nw])
                ch = chpool.tile([P, maxc * D], dt.bfloat16, tag="msgs")
                nc.sync.dma_start(
                    out=ch[:, :(c1 - c0) * D],
                    in_=msgs_d.ap()[:, c0 * D:c1 * D])
                chunk_tiles[ci] = ch

            for w0, sw in supers:
                agg_ps = psA.tile([D, 4 * P], dt.float32, tag="agg")
                for s in range(sw):
                    wi = w0 + s
                    ci = wi // CHUNK_W
                    ch = chunk_tiles[ci]
                    kw = int(Kw[wi])
                    for k in range(kw):
                        t = int(offw[wi]) + k
                        rel = t - int(offw[ci * CHUNK_W])
                        mt = mpool.tile([P, P], dt.bfloat16, tag="M")
                        nc.vector.tensor_scalar(
                            out=mt[:], in0=iota_sb[:],
                            scalar1=dstloc_sb[:, t:t + 1],
                            scalar2=wts_sb[:, t:t + 1],
                            op0=mybir.AluOpType.is_equal,
                            op1=mybir.AluOpType.mult)
                        nc.tensor.matmul(
                            out=agg_ps[:, s * P:(s + 1) * P],
                            lhsT=ch[:, rel * D:(rel + 1) * D],
                            rhs=mt[:], start=(k == 0), stop=(k == kw - 1))
                agg_sb = spool.tile([D, 4 * P], dt.bfloat16, tag="aggsb")
                nc.vector.tensor_copy(out=agg_sb[:, :sw * P],
                                      in_=agg_ps[:, :sw * P])
                h_ps = psB.tile([D, 4 * P], dt.float32, tag="hps")
                nc.tensor.matmul(out=h_ps[:, :sw * P], lhsT=w1l_sb[:],
                                 rhs=agg_sb[:, :sw * P], start=True, stop=False)
                nc.tensor.matmul(out=h_ps[:, :sw * P], lhsT=w1r_sb[:],
                                 rhs=xT_sb[:, w0 * P:(w0 + sw) * P],
                                 start=False, stop=True)
                nc.scalar.activation(
                    out=h1T_sb[:, w0 * P:(w0 + sw) * P], in_=h_ps[:, :sw * P],
                    func=mybir.ActivationFunctionType.Relu, bias=b1_sb[:])
                # transpose this super's windows into row staging
                for wi in range(w0, w0 + sw):
                    tp = psT.tile([P, D], dt.float32, tag="tp")
                    nc.tensor.matmul(out=tp[:],
                                     lhsT=h1T_sb[:, wi * P:(wi + 1) * P],
                                     rhs=id64_sb[:], start=True, stop=True)
                    nc.vector.tensor_copy(out=h1rows_sb[:, wi * D:(wi + 1) * D],
                                          in_=tp[:])
                # chunk complete -> stage to DRAM + AllGather (overlaps L1)
                if (w0 + sw) % AG_W == 0:
                    g = (w0 + sw) // AG_W - 1
                    nc.sync.dma_start(
                        out=h1loc_dram[g * AG_ROWS:(g + 1) * AG_ROWS]
                        .rearrange("(p t) f -> p (t f)", p=P),
                        in_=h1rows_sb[:, g * AG_W * D:(g + 1) * AG_W * D])
                    nc.gpsimd.collective_compute(
                        "AllGather", mybir.AluOpType.bypass,
                        replica_groups=[list(range(NCORES))],
                        ins=[h1loc_dram[g * AG_ROWS:(g + 1) * AG_ROWS]],
                        outs=[h1full_dram[g * NCORES * AG_ROWS:
                                          (g + 1) * NCORES * AG_ROWS]])

            # ---------------- layer 2 ----------------
            out2T_sb = cpool.tile([D, WROWS], dt.float32, tag="out2T")
            for w0, sw in supers:
                agg_ps = psA.tile([D, 4 * P], dt.float32, tag="agg")
                for s in range(sw):
                    wi = w0 + s
                    kw = int(Kw[wi])
                    for k in range(kw):
                        t = int(offw[wi]) + k
                        gt = gpool.tile([P, D], dt.bfloat16, tag="g")
                        nc.gpsimd.indirect_dma_start(
                            out=gt[:], out_offset=None, in_=h1full_dram[:],
                            in_offset=bass.IndirectOffsetOnAxis(
                                ap=src2_sb[:, t:t + 1], axis=0))
                        mt = mpool.tile([P, P], dt.bfloat16, tag="M")
                        nc.vector.tensor_scalar(
                            out=mt[:], in0=iota_sb[:],
                            scalar1=dstloc_sb[:, t:t + 1],
                            scalar2=wts_sb[:, t:t + 1],
                            op0=mybir.AluOpType.is_equal,
                            op1=mybir.AluOpType.mult)
                        nc.tensor.matmul(
                            out=agg_ps[:, s * P:(s + 1) * P], lhsT=gt[:],
                            rhs=mt[:], start=(k == 0), stop=(k == kw - 1))
                agg_sb = spool.tile([D, 4 * P], dt.bfloat16, tag="aggsb")
                nc.vector.tensor_copy(out=agg_sb[:, :sw * P],
                                      in_=agg_ps[:, :sw * P])
                h_ps = psB.tile([D, 4 * P], dt.float32, tag="hps")
                nc.tensor.matmul(out=h_ps[:, :sw * P], lhsT=w2l_sb[:],
                                 rhs=agg_sb[:, :sw * P], start=True, stop=False)
                nc.tensor.matmul(out=h_ps[:, :sw * P], lhsT=w2r_sb[:],
                                 rhs=h1T_sb[:, w0 * P:(w0 + sw) * P],
                                 start=False, stop=True)
                nc.vector.tensor_scalar_add(
                    out=out2T_sb[:, w0 * P:(w0 + sw) * P],
                    in0=h_ps[:, :sw * P], scalar1=b2_sb[:])

            # transpose out2T -> rows -> DRAM
            outv = out_d.ap().rearrange("(p t) f -> p t f", p=P)
            for wi in range(NW):
                tp = psT.tile([P, D], dt.float32, tag="tp")
                nc.tensor.matmul(out=tp[:], lhsT=out2T_sb[:, wi * P:(wi + 1) * P],
                                 rhs=id64f_sb[:], start=True, stop=True)
                ot = spool.tile([P, D], dt.float32, tag="orow")
                nc.vector.tensor_copy(out=ot[:], in_=tp[:])
                nc.sync.dma_start(out=outv[:, wi, :], in_=ot[:])

    nc.compile()
    return nc


_CACHE = {}


def kernel(x, edge_index, W1l, W1r, b1, W2l, W2r, b2):
    from concourse import bass_utils

    x = np.asarray(x, dtype=np.float32)
    edge_index = np.asarray(edge_index)
    src = edge_index[0].astype(np.int64)
    dst = edge_index[1].astype(np.int64)
    cnt = np.bincount(dst, minlength=N).astype(np.float32)
    inv = (1.0 / np.maximum(cnt, 1.0)).astype(np.float32)

    # per-window tile count (max across cores, SPMD-uniform)
    wid = (dst % NLOC) // P + (dst // NLOC) * NW
    wc = np.bincount(wid, minlength=NCORES * NW)
    Kw = np.maximum((wc.reshape(NCORES, NW).max(axis=0) + P - 1) // P,
                    1).astype(np.int64)
    offw = np.concatenate([[0], np.cumsum(Kw)]).astype(np.int64)

    iota = np.tile(np.arange(P, dtype=np.float32), (P, 1)).astype(BF16)
    id64 = np.eye(D, dtype=np.float32)
    common = {
        "iota": iota, "id64": id64.astype(BF16), "id64f": id64,
        "w1lT": np.asarray(W1l, np.float32).T.astype(BF16).copy(),
        "w1rT": np.asarray(W1r, np.float32).T.astype(BF16).copy(),
        "w2lT": np.asarray(W2l, np.float32).T.astype(BF16).copy(),
        "w2rT": np.asarray(W2r, np.float32).T.astype(BF16).copy(),
        "b1c": np.asarray(b1, np.float32).reshape(D, 1).copy(),
        "b2c": np.asarray(b2, np.float32).reshape(D, 1).copy(),
    }
    in_maps = []
    for c in range(NCORES):
        msgs_pt, dstloc_pt, w_pt, src2_pt, xT = _prep_core(
            c, src, dst, inv, x, Kw, offw)
        in_maps.append({**common, "msgs": msgs_pt, "dstloc": dstloc_pt,
                        "wts": w_pt, "src2": src2_pt, "xT": xT})

    nc = _build_program(Kw, offw)
    _CACHE['all'] = (nc, in_maps)
    res = bass_utils.run_bass_kernel_spmd(nc, in_maps, list(range(NCORES)))

    outs = []
    for c in range(NCORES):
        o = res.results[c]["out"]  # [WROWS, 64], row = p*NW + t
        o = o.reshape(P, NW, D).transpose(1, 0, 2).reshape(WROWS, D)[:NLOC]
        outs.append(o)
    return np.concatenate(outs, axis=0).astype(np.float32)

